# revision 1
# baseline (speedup 1.0000x reference)
"""GAT layer (PyG-style, concat=False) on 8 Trainium2 NeuronCores.

Sharding: one attention head per core (H == n_cores == 8). Each core:
  phase 1: h = x @ W_head (bf16 PE matmul), a_src/a_dst matvecs; writes a
           768B-per-node table h_ext[N, 384] = [h(256)|a_src|a_dst|1.0|pad].
  phase 2: edges grouped by 128-row dst tiles; per 128-edge chunk, dma_gather
           fetches the src rows and the dst score rows, scores go through
           Prelu(0.2)+Exp, a fused DVE op builds the exp-scaled one-hot, and
           one PE matmul scatter-accumulates messages + denominator into PSUM.
           Per tile: divide by (denom + eps), DMA out.
Host averages the 8 per-head outputs and adds bias. No collectives.
"""

import numpy as np
import ml_dtypes

import concourse.bass as bass
import concourse.bacc as bacc
import concourse.mybir as mybir
from concourse.tile import TileContext
from concourse.bass_utils import run_bass_kernel_spmd

N = 50000
E = 200000
H = 8
C = 256
IN = 256
NEG_SLOPE = 0.2
EPS = 1e-16

P = 128
NT = (N + P - 1) // P            # 391 dst tiles (last has 80 rows)
ROW = 384                        # h_ext row width (bf16) -> 768B
SCOFF = 256                      # score columns start (a_src, a_dst, one)
B = 32                           # chunks per gather batch
NIDX = B * P                     # indices per batch (4096)
HI_OFF = 17232                   # high-table row offset (N-1-HI_OFF <= 32767)
BF16 = ml_dtypes.bfloat16


def _wrap16(ix):
    """[NIDX] int -> [128, NIDX//16] int16 wrapped in 16 partitions, x8 replicated."""
    a = ix.reshape(-1, 16).T.astype(np.int16)
    return np.tile(a, (8, 1))


def _preprocess(edge_index):
    """Build chunk/batch structures shared by all cores.

    Returns dict with:
      idxh  [128, NB*NIDX//16] int16  row-gather indices per batch (wrapped)
      idxs  [128, NB*NIDX//16] int16  score-gather indices per batch (wrapped)
      dstl  [128, NB*B] f32           local dst per chunk slot (-1 = pad)
      batches: list of (src_hi, dst_hi)
      events: list of ('batch', b) / ('tile', t, nr, [(b, slot), ...])
    """
    src = edge_index[0].astype(np.int64)
    dst = edge_index[1].astype(np.int64)
    order = np.argsort(dst, kind="stable")
    dst_sorted = dst[order]
    tile_starts = np.searchsorted(dst_sorted, np.arange(0, NT * P + 1, P))

    # --- build chunks per tile (tile-major order) ---
    # chunk record: (tile, eids (np array, may be empty), src_hi)
    chunks = []
    tile_chunk_ids = [[] for _ in range(NT)]
    for t in range(NT):
        lo_, hi_ = tile_starts[t], tile_starts[t + 1]
        eids = order[lo_:hi_]
        if len(eids):
            eids = eids[np.argsort(src[eids], kind="stable")]
            s = src[eids]
            cut = int(np.searchsorted(s, 32768))
            parts = [(eids[:cut], False), (eids[cut:], True)]
        else:
            parts = [(eids, False)]  # ensure >=1 chunk to zero the PSUM
        got = False
        for part, shi in parts:
            if len(part) == 0 and got:
                continue
            if len(part) == 0:
                tile_chunk_ids[t].append(len(chunks))
                chunks.append((t, part, shi))
                got = True
                continue
            for i in range(0, len(part), P):
                tile_chunk_ids[t].append(len(chunks))
                chunks.append((t, part[i : i + P], shi))
                got = True

    # --- assign chunks to class-pure batches of B, emit events ---
    batches = []        # (src_hi, dst_hi)
    batch_slots = []    # list per batch: list of chunk ids (or -1 pad)
    open_batches = {}   # (src_hi, dst_hi) -> batch idx
    chunk_pos = {}      # chunk id -> (batch, slot)
    closed = set()
    events = []
    tiles_pending = []  # tiles fully assigned, waiting for batch closure
    emitted_tiles = set()

    def close_batch(bi):
        while len(batch_slots[bi]) < B:
            batch_slots[bi].append(-1)
        closed.add(bi)
        events.append(("batch", bi))
        # emit tiles that became ready
        still = []
        for t in tiles_pending:
            if all(chunk_pos[c][0] in closed for c in tile_chunk_ids[t]):
                nr = min(P, N - t * P)
                events.append(
                    ("tile", t, nr, [chunk_pos[c] for c in tile_chunk_ids[t]])
                )
                emitted_tiles.add(t)
            else:
                still.append(t)
        tiles_pending[:] = still

    cur_dst_hi = False
    for t in range(NT):
        dst_hi = t >= 256
        if dst_hi and not cur_dst_hi:
            # dst-class boundary: close all open dst-lo batches
            for key in list(open_batches):
                close_batch(open_batches.pop(key))
            cur_dst_hi = True
        for c in tile_chunk_ids[t]:
            _, _, shi = chunks[c]
            key = (shi, dst_hi)
            if key not in open_batches:
                batches.append(key)
                batch_slots.append([])
                open_batches[key] = len(batches) - 1
            bi = open_batches[key]
            chunk_pos[c] = (bi, len(batch_slots[bi]))
            batch_slots[bi].append(c)
            if len(batch_slots[bi]) == B:
                del open_batches[key]
                close_batch(bi)
        tiles_pending.append(t)
    for key in list(open_batches):
        close_batch(open_batches.pop(key))
    assert not tiles_pending and len(emitted_tiles) == NT

    # --- build index arrays ---
    NB = len(batches)
    idxh = np.zeros((128, NB * (NIDX // 16)), np.int16)
    idxs = np.zeros((128, NB * (NIDX // 16)), np.int16)
    dstl = np.full((128, NB * B), -1.0, np.float32)
    for bi, (shi, dhi) in enumerate(batches):
        hix = np.zeros(NIDX, np.int64)
        six = np.zeros(NIDX, np.int64)
        for s_i, c in enumerate(batch_slots[bi]):
            if c < 0:
                continue
            t, eids, c_shi = chunks[c]
            ne = len(eids)
            if ne:
                sv = src[eids] - (HI_OFF if c_shi else 0)
                dv = dst[eids] - (HI_OFF if dhi else 0)
                hix[s_i * P : s_i * P + ne] = sv
                six[s_i * P : s_i * P + ne] = dv
                dstl[:ne, bi * B + s_i] = (dst[eids] - t * P).astype(np.float32)
        idxh[:, bi * (NIDX // 16) : (bi + 1) * (NIDX // 16)] = _wrap16(hix)
        idxs[:, bi * (NIDX // 16) : (bi + 1) * (NIDX // 16)] = _wrap16(six)

    return {
        "idxh": idxh,
        "idxs": idxs,
        "dstl": dstl,
        "batches": batches,
        "events": events,
    }


def _build_program(pp, timing=False, variant="full", repeat=1):
    """Build the per-core Bacc program (identical for all cores).

    timing=True: external output is a tiny [P, C] tensor and per-tile results go
    to an internal DRAM tensor instead — removes host<->device transfer noise
    when benchmarking; compute/DMA work is otherwise identical.
    variant: 'full' | 'phase1' | 'gather' (timing ablations).
    """
    NB = len(pp["batches"])
    nc = bacc.Bacc()
    bf = mybir.dt.bfloat16
    f32 = mybir.dt.float32

    t_xT = nc.declare_dram_parameter("xT", [IN, N], bf, isOutput=False)
    t_W = nc.declare_dram_parameter("Wh", [IN, C], bf, isOutput=False)
    t_wsd = nc.declare_dram_parameter("wsd", [IN, 2], bf, isOutput=False)
    t_iota = nc.declare_dram_parameter("iota", [P, P], f32, isOutput=False)
    t_idxh = nc.declare_dram_parameter("idxh", [128, NB * (NIDX // 16)], mybir.dt.int16, isOutput=False)
    t_idxs = nc.declare_dram_parameter("idxs", [128, NB * (NIDX // 16)], mybir.dt.int16, isOutput=False)
    t_dstl = nc.declare_dram_parameter("dstl", [128, NB * B], f32, isOutput=False)
    if timing:
        t_out = nc.declare_dram_parameter("out", [P, C], f32, isOutput=True)
        out_dst = nc.dram_tensor("out_int", [N, C], f32)
    else:
        t_out = nc.declare_dram_parameter("out", [N, C], f32, isOutput=True)
        out_dst = t_out
    h_ext = nc.dram_tensor("h_ext", [N, ROW], bf)
    sc_tab = nc.dram_tensor("sc_tab", [N, 128], bf)

    with TileContext(nc) as tc:
        with (
            tc.tile_pool(name="const", bufs=1) as cpool,
            tc.tile_pool(name="xa", bufs=4) as xa,
            tc.tile_pool(name="hs", bufs=3) as hs,
            tc.tile_pool(name="ph", bufs=2, space="PSUM") as ph,
            tc.tile_pool(name="pa", bufs=2, space="PSUM") as pa,
        ):
            iota_t = cpool.tile([P, P], f32)
            nc.sync.dma_start(out=iota_t[:], in_=t_iota[:])
            w0 = cpool.tile([128, C], bf, tag="w0")
            w1 = cpool.tile([128, C], bf, tag="w1")
            nc.sync.dma_start(out=w0[:], in_=t_W[0:128, :])
            nc.sync.dma_start(out=w1[:], in_=t_W[128:256, :])
            wsd0 = cpool.tile([128, 2], bf, tag="wsd0")
            wsd1 = cpool.tile([128, 2], bf, tag="wsd1")
            nc.sync.dma_start(out=wsd0[:], in_=t_wsd[0:128, :])
            nc.sync.dma_start(out=wsd1[:], in_=t_wsd[128:256, :])

            # ---------------- phase 1: h_ext = [x@W | a_src | a_dst | 1] ----
            for _rep in range(repeat):
              if _rep > 0:
                tc.strict_bb_all_engine_barrier()
              for t in range(NT):
                n0 = t * P
                nr = min(P, N - n0)
                xt0 = xa.tile([128, P], bf, tag="xt0")
                xt1 = xa.tile([128, P], bf, tag="xt1")
                nc.sync.dma_start(out=xt0[:, :nr], in_=t_xT[0:128, n0 : n0 + nr])
                nc.sync.dma_start(out=xt1[:, :nr], in_=t_xT[128:256, n0 : n0 + nr])
                ph_t = ph.tile([P, C], f32, space="PSUM")
                nc.tensor.matmul(out=ph_t[:nr, :], lhsT=xt0[:, :nr], rhs=w0[:], start=True, stop=False)
                nc.tensor.matmul(out=ph_t[:nr, :], lhsT=xt1[:, :nr], rhs=w1[:], start=False, stop=True)
                pa_t = pa.tile([P, 2], f32, space="PSUM")
                nc.tensor.matmul(out=pa_t[:nr, :], lhsT=xt0[:, :nr], rhs=wsd0[:], start=True, stop=False)
                nc.tensor.matmul(out=pa_t[:nr, :], lhsT=xt1[:, :nr], rhs=wsd1[:], start=False, stop=True)
                h_sb = hs.tile([P, ROW], bf, tag="hsb")
                nc.vector.tensor_copy(out=h_sb[:nr, 0:C], in_=ph_t[:nr, :])
                nc.vector.tensor_copy(out=h_sb[:nr, SCOFF : SCOFF + 2], in_=pa_t[:nr, :])
                nc.vector.memset(h_sb[:nr, SCOFF + 2 : SCOFF + 3], 1.0)
                nc.sync.dma_start(out=h_ext[n0 : n0 + nr, :], in_=h_sb[:nr, :])
                sc_sb = hs.tile([P, 128], bf, tag="scsb")
                nc.vector.tensor_copy(out=sc_sb[:nr, 0:2], in_=pa_t[:nr, :])
                nc.sync.dma_start(out=sc_tab[n0 : n0 + nr, :], in_=sc_sb[:nr, :])

              tc.strict_bb_all_engine_barrier()

              # ---------------- phase 2: gather / softmax / scatter --------
              if variant != "phase1":
                  _phase2(nc, tc, pp, iota_t, t_idxh, t_idxs, t_dstl, h_ext, sc_tab, t_out, out_dst, variant)

    nc.finalize()
    return nc


def _phase2(nc, tc, pp, iota_t, t_idxh, t_idxs, t_dstl, h_ext, sc_tab, t_out, out_dst, variant):
    bf = mybir.dt.bfloat16
    f32 = mybir.dt.float32
    if True:
            with (
                tc.tile_pool(name="gb", bufs=4) as gb,
                tc.tile_pool(name="ib", bufs=4) as ib,
                tc.tile_pool(name="scp", bufs=4) as scp,
                tc.tile_pool(name="ohp", bufs=4) as ohp,
                tc.tile_pool(name="po", bufs=4, space="PSUM") as po,
                tc.tile_pool(name="ou", bufs=3) as ou,
            ):
                g_tiles = {}
                e_tiles = {}
                d_tiles = {}
                for ev in pp["events"]:
                    if ev[0] == "batch":
                        bi = ev[1]
                        shi, dhi = pp["batches"][bi]
                        ih = ib.tile([128, NIDX // 16], mybir.dt.int16, tag="ih")
                        is_ = ib.tile([128, NIDX // 16], mybir.dt.int16, tag="is")
                        dl = ib.tile([128, B], f32, tag="dl")
                        c0 = bi * (NIDX // 16)
                        nc.sync.dma_start(out=ih[:], in_=t_idxh[:, c0 : c0 + NIDX // 16])
                        nc.sync.dma_start(out=is_[:], in_=t_idxs[:, c0 : c0 + NIDX // 16])
                        nc.sync.dma_start(out=dl[:], in_=t_dstl[:, bi * B : (bi + 1) * B])
                        g_t = gb.tile([P, B * ROW], bf, tag="g")
                        s_t = gb.tile([P, B * 128], bf, tag="s")
                        tab = h_ext[HI_OFF:, :] if shi else h_ext[:, :]
                        stab = sc_tab[HI_OFF:, :] if dhi else sc_tab[:, :]
                        QN = 1024
                        for q in range(NIDX // QN):
                            qsl = slice(q * (QN // 16), (q + 1) * (QN // 16))
                            gsl = slice(q * (QN // P) * ROW, (q + 1) * (QN // P) * ROW)
                            ssl = slice(q * (QN // P) * 128, (q + 1) * (QN // P) * 128)
                            nc.gpsimd.dma_gather(
                                g_t[:, gsl].rearrange("p (c e) -> p c e", e=ROW),
                                tab, ih[:, qsl], QN, QN, ROW,
                                single_packet=True,
                            )
                            nc.gpsimd.dma_gather(
                                s_t[:, ssl].rearrange("p (c e) -> p c e", e=128),
                                stab, is_[:, qsl], QN, QN, 128,
                                single_packet=True,
                            )
                        g3 = g_t[:].rearrange("p (c e) -> p c e", e=ROW)
                        s3 = s_t[:].rearrange("p (c e) -> p c e", e=128)
                        ss = scp.tile([P, B], f32, tag="ss")
                        se = scp.tile([P, B], f32, tag="se")
                        nc.vector.tensor_tensor(
                            out=ss[:].rearrange("p (c e) -> p c e", e=1),
                            in0=g3[:, :, SCOFF : SCOFF + 1],
                            in1=s3[:, :, 1:2],
                            op=mybir.AluOpType.add,
                        )
                        nc.scalar.activation(out=ss[:], in_=ss[:], func=mybir.ActivationFunctionType.Prelu, alpha=NEG_SLOPE)
                        nc.scalar.activation(out=se[:], in_=ss[:], func=mybir.ActivationFunctionType.Exp)
                        g_tiles[bi] = g_t
                        e_tiles[bi] = se
                        d_tiles[bi] = dl
                        if variant == "gather":
                            jk = ou.tile([P, 4], f32, tag="junk")
                            nc.vector.tensor_copy(out=jk[:], in_=g_t[:, 0:4])
                            nc.vector.tensor_copy(out=jk[:, 0:1], in_=se[:, 0:1])
                            nc.sync.dma_start(out=out_dst[0:P, 0:4], in_=jk[:])
                    elif variant == "gather":
                        continue
                    else:
                        _, t, nr, slots = ev
                        pt = po.tile([P, C + 3], f32, space="PSUM")
                        nch = len(slots)
                        for j, (bi, s) in enumerate(slots):
                            oh_t = ohp.tile([P, P], bf, tag="oh")
                            nc.vector.tensor_scalar(
                                out=oh_t[:],
                                in0=iota_t[:],
                                scalar1=d_tiles[bi][:, s : s + 1],
                                scalar2=e_tiles[bi][:, s : s + 1],
                                op0=mybir.AluOpType.is_equal,
                                op1=mybir.AluOpType.mult,
                            )
                            nc.tensor.matmul(
                                out=pt[:, :],
                                lhsT=oh_t[:],
                                rhs=g_tiles[bi][:, s * ROW : s * ROW + C + 3],
                                start=(j == 0),
                                stop=(j == nch - 1),
                            )
                        dn = ou.tile([P, 1], f32, tag="dn")
                        nc.vector.tensor_scalar_add(out=dn[:], in0=pt[:, C + 2 : C + 3], scalar1=EPS)
                        rc = ou.tile([P, 1], f32, tag="rc")
                        nc.vector.reciprocal(out=rc[:], in_=dn[:])
                        ob = ou.tile([P, C], f32, tag="ob")
                        nc.vector.tensor_scalar_mul(out=ob[:], in0=pt[:, 0:C], scalar1=rc[:, :1])
                        nc.sync.dma_start(out=out_dst[t * P : t * P + nr, :], in_=ob[:nr, :])
                if out_dst is not t_out:
                    tc.strict_bb_all_engine_barrier()
                    fin = ou.tile([P, C], f32, tag="fin")
                    nc.sync.dma_start(out=fin[:], in_=out_dst[0:P, :])
                    nc.sync.dma_start(out=t_out[:, :], in_=fin[:])


def _make_in_maps(x, W, att_src, att_dst, pp):
    xT = np.ascontiguousarray(x.astype(BF16).T)
    iota = np.broadcast_to(np.arange(P, dtype=np.float32), (P, P)).copy()
    base = {
        "xT": xT,
        "iota": iota,
        "idxh": pp["idxh"],
        "idxs": pp["idxs"],
        "dstl": pp["dstl"],
    }
    in_maps = []
    for h in range(H):
        Wh = W[:, h * C : (h + 1) * C].astype(np.float32)
        wsrc = Wh @ att_src[h].astype(np.float32)
        wdst = Wh @ att_dst[h].astype(np.float32)
        m = dict(base)
        m["Wh"] = Wh.astype(BF16)
        m["wsd"] = np.stack([wsrc, wdst], axis=1).astype(BF16)
        in_maps.append(m)
    return in_maps


_CACHE = {}


def _get_compiled(edge_index):
    key = edge_index.tobytes()
    ck = _CACHE.get("key")
    if ck != key:
        pp = _preprocess(edge_index)
        nc = _build_program(pp)
        _CACHE.update(key=key, pp=pp, nc=nc)
    return _CACHE["pp"], _CACHE["nc"]


def kernel(x, edge_index, W, att_src, att_dst, bias, _timing=None):
    x = np.asarray(x)
    edge_index = np.asarray(edge_index)
    W = np.asarray(W)
    att_src = np.asarray(att_src)
    att_dst = np.asarray(att_dst)
    bias = np.asarray(bias)

    pp, nc = _get_compiled(edge_index)
    in_maps = _make_in_maps(x, W, att_src, att_dst, pp)
    res = run_bass_kernel_spmd(nc, in_maps, core_ids=list(range(H)))
    if _timing is not None:
        _timing["exec_time_ns"] = res.exec_time_ns
    acc = np.zeros((N, C), np.float64)
    for h in range(H):
        acc += res.results[h]["out"].astype(np.float64)
    out = (acc / H) + bias.astype(np.float64)
    return out.astype(np.float32)



# revision 3
# speedup vs baseline: 8.1871x; 8.1871x over previous
"""GAT layer (PyG-style, concat=False) on 8 Trainium2 NeuronCores.

Sharding: one attention head per core (H == n_cores == 8), with all large
host<->device traffic minimized (the axon tunnel runs at ~35 MB/s, so wire
bytes dominate wall time):
  - x is sent SHARDED: core c gets rows [c*6250, (c+1)*6250) as bf16 and the
    full [N, IN] table is rebuilt on-device with an AllGather collective.
  - edge index tables are sent compact ([16, .] int16, not replicated to 128
    partitions; dst-locals as bf16) and expanded on-device.
  - each core computes its head's output, scales by 1/8, and a
    ReduceScatter(add) leaves each core with a [6250, C] shard of the final
    head-mean; only that shard (float16) is returned to the host.

Per-core program:
  phase 0: AllGather x shards -> x_full [N, IN] bf16 (internal DRAM).
  phase 1: per 128-node tile: load x rows, PE-transpose, h = x @ W_head
           (bf16 PE matmul), a_src/a_dst matvecs; writes a 768B-per-node
           table h_ext[N, 384] = [h(256)|a_src|a_dst|1.0|pad] + score table.
  phase 2: edges grouped by 128-row dst tiles; per 128-edge chunk, dma_gather
           fetches src rows and dst score rows, scores go through
           Prelu(0.2)+Exp, a fused DVE op builds the exp-scaled one-hot, and
           one PE matmul scatter-accumulates messages + denominator into
           PSUM. Per tile: multiply by 1/(8*(denom+eps)), DMA to rs_in.
  phase 3: ReduceScatter(add) rs_in -> [6250, C] shard; cast f16; DMA out.
Host concatenates the 8 shards and adds bias.
"""

import numpy as np
import ml_dtypes

import concourse.bass as bass
import concourse.bacc as bacc
import concourse.mybir as mybir
from concourse.tile import TileContext
from concourse.bass_utils import run_bass_kernel_spmd

N = 50000
E = 200000
H = 8
C = 256
IN = 256
NEG_SLOPE = 0.2
EPS = 1e-16

P = 128
NT = (N + P - 1) // P            # 391 dst tiles (last has 80 rows)
NS = N // H                      # 6250 output rows per core
NTS = (NS + P - 1) // P          # 49 readback tiles (last has 106 rows)
ROW = 384                        # h_ext row width (bf16) -> 768B
SCOFF = 256                      # score columns start (a_src, a_dst, one)
B = 32                           # chunks per gather batch
NIDX = B * P                     # indices per batch (4096)
HI_OFF = 17232                   # high-table row offset (N-1-HI_OFF <= 32767)
BF16 = ml_dtypes.bfloat16


def _wrap16(ix):
    """[NIDX] int -> [16, NIDX//16] int16 (16-partition wrapped, compact)."""
    return ix.reshape(-1, 16).T.astype(np.int16)


def _preprocess(edge_index):
    """Build chunk/batch structures shared by all cores.

    Returns dict with:
      idxh  [16, NB*NIDX//16] int16   row-gather indices per batch (wrapped)
      idxs  [16, NB*NIDX//16] int16   score-gather indices per batch (wrapped)
      dstl  [128, NB*B] bf16          local dst per chunk slot (-1 = pad)
      batches: list of (src_hi, dst_hi)
      events: list of ('batch', b) / ('tile', t, nr, [(b, slot), ...])
    """
    src = edge_index[0].astype(np.int64)
    dst = edge_index[1].astype(np.int64)
    order = np.argsort(dst, kind="stable")
    dst_sorted = dst[order]
    tile_starts = np.searchsorted(dst_sorted, np.arange(0, NT * P + 1, P))

    # --- build chunks per tile (tile-major order) ---
    chunks = []
    tile_chunk_ids = [[] for _ in range(NT)]
    for t in range(NT):
        lo_, hi_ = tile_starts[t], tile_starts[t + 1]
        eids = order[lo_:hi_]
        if len(eids):
            eids = eids[np.argsort(src[eids], kind="stable")]
            s = src[eids]
            cut = int(np.searchsorted(s, 32768))
            parts = [(eids[:cut], False), (eids[cut:], True)]
        else:
            parts = [(eids, False)]  # ensure >=1 chunk to zero the PSUM
        got = False
        for part, shi in parts:
            if len(part) == 0 and got:
                continue
            if len(part) == 0:
                tile_chunk_ids[t].append(len(chunks))
                chunks.append((t, part, shi))
                got = True
                continue
            for i in range(0, len(part), P):
                tile_chunk_ids[t].append(len(chunks))
                chunks.append((t, part[i : i + P], shi))
                got = True

    # --- assign chunks to class-pure batches of B, emit events ---
    batches = []        # (src_hi, dst_hi)
    batch_slots = []    # list per batch: list of chunk ids (or -1 pad)
    open_batches = {}   # (src_hi, dst_hi) -> batch idx
    chunk_pos = {}      # chunk id -> (batch, slot)
    closed = set()
    events = []
    tiles_pending = []
    emitted_tiles = set()

    def close_batch(bi):
        while len(batch_slots[bi]) < B:
            batch_slots[bi].append(-1)
        closed.add(bi)
        events.append(("batch", bi))
        still = []
        for t in tiles_pending:
            if all(chunk_pos[c][0] in closed for c in tile_chunk_ids[t]):
                nr = min(P, N - t * P)
                events.append(
                    ("tile", t, nr, [chunk_pos[c] for c in tile_chunk_ids[t]])
                )
                emitted_tiles.add(t)
            else:
                still.append(t)
        tiles_pending[:] = still

    cur_dst_hi = False
    for t in range(NT):
        dst_hi = t >= 256
        if dst_hi and not cur_dst_hi:
            for key in list(open_batches):
                close_batch(open_batches.pop(key))
            cur_dst_hi = True
        for c in tile_chunk_ids[t]:
            _, _, shi = chunks[c]
            key = (shi, dst_hi)
            if key not in open_batches:
                batches.append(key)
                batch_slots.append([])
                open_batches[key] = len(batches) - 1
            bi = open_batches[key]
            chunk_pos[c] = (bi, len(batch_slots[bi]))
            batch_slots[bi].append(c)
            if len(batch_slots[bi]) == B:
                del open_batches[key]
                close_batch(bi)
        tiles_pending.append(t)
    for key in list(open_batches):
        close_batch(open_batches.pop(key))
    assert not tiles_pending and len(emitted_tiles) == NT

    # --- build compact index arrays ---
    NB = len(batches)
    idxh = np.zeros((16, NB * (NIDX // 16)), np.int16)
    idxs = np.zeros((16, NB * (NIDX // 16)), np.int16)
    dstl = np.full((128, NB * B), -1.0, BF16)
    for bi, (shi, dhi) in enumerate(batches):
        hix = np.zeros(NIDX, np.int64)
        six = np.zeros(NIDX, np.int64)
        for s_i, c in enumerate(batch_slots[bi]):
            if c < 0:
                continue
            t, eids, c_shi = chunks[c]
            ne = len(eids)
            if ne:
                sv = src[eids] - (HI_OFF if c_shi else 0)
                dv = dst[eids] - (HI_OFF if dhi else 0)
                hix[s_i * P : s_i * P + ne] = sv
                six[s_i * P : s_i * P + ne] = dv
                dstl[:ne, bi * B + s_i] = (dst[eids] - t * P).astype(BF16)
        idxh[:, bi * (NIDX // 16) : (bi + 1) * (NIDX // 16)] = _wrap16(hix)
        idxs[:, bi * (NIDX // 16) : (bi + 1) * (NIDX // 16)] = _wrap16(six)

    return {
        "idxh": idxh,
        "idxs": idxs,
        "dstl": dstl,
        "batches": batches,
        "events": events,
    }


def _build_program(pp):
    """Build the per-core Bacc program (identical for all cores)."""
    NB = len(pp["batches"])
    nc = bacc.Bacc(num_devices=8)
    bf = mybir.dt.bfloat16
    f16 = mybir.dt.float16
    f32 = mybir.dt.float32
    i16 = mybir.dt.int16
    GRP = [list(range(8))]

    t_xsl = nc.declare_dram_parameter("xsl", [NS, IN], bf, isOutput=False)
    t_W = nc.declare_dram_parameter("Wh", [IN, C], bf, isOutput=False)
    t_wsd = nc.declare_dram_parameter("wsd", [IN, 2], bf, isOutput=False)
    t_iota = nc.declare_dram_parameter("iota", [P, P], f32, isOutput=False)
    t_ident = nc.declare_dram_parameter("ident", [P, P], bf, isOutput=False)
    t_idxh = nc.declare_dram_parameter("idxh", [16, NB * (NIDX // 16)], i16, isOutput=False)
    t_idxs = nc.declare_dram_parameter("idxs", [16, NB * (NIDX // 16)], i16, isOutput=False)
    t_dstl = nc.declare_dram_parameter("dstl", [128, NB * B], bf, isOutput=False)
    t_out = nc.declare_dram_parameter("out", [NS, C], f16, isOutput=True)

    h_ext = nc.dram_tensor("h_ext", [N, ROW], bf)
    sc_tab = nc.dram_tensor("sc_tab", [N, 128], bf)

    with TileContext(nc) as tc:
        with (
            tc.tile_pool(name="dramp", bufs=1, space="DRAM") as dramp,
            tc.tile_pool(name="const", bufs=1) as cpool,
            tc.tile_pool(name="xa", bufs=4) as xa,
            tc.tile_pool(name="hs", bufs=3) as hs,
            tc.tile_pool(name="ph", bufs=2, space="PSUM") as ph,
            tc.tile_pool(name="pa", bufs=2, space="PSUM") as pa,
        ):
            x_bounce = dramp.tile([NS, IN], bf)
            x_full = dramp.tile([N, IN], bf)
            rs_in = dramp.tile([N, C], f32)
            rs_out = dramp.tile([NS, C], f32)

            # ---------------- phase 0: AllGather x shards -------------------
            nc.gpsimd.dma_start(x_bounce[:], t_xsl[:])
            nc.gpsimd.collective_compute(
                "AllGather", mybir.AluOpType.bypass,
                replica_groups=GRP, ins=[x_bounce.opt()], outs=[x_full.opt()],
            )

            iota_t = cpool.tile([P, P], f32)
            nc.sync.dma_start(out=iota_t[:], in_=t_iota[:])
            ident_t = cpool.tile([P, P], bf)
            nc.sync.dma_start(out=ident_t[:], in_=t_ident[:])
            w0 = cpool.tile([128, C], bf, tag="w0")
            w1 = cpool.tile([128, C], bf, tag="w1")
            nc.sync.dma_start(out=w0[:], in_=t_W[0:128, :])
            nc.sync.dma_start(out=w1[:], in_=t_W[128:256, :])
            wsd0 = cpool.tile([128, 2], bf, tag="wsd0")
            wsd1 = cpool.tile([128, 2], bf, tag="wsd1")
            nc.sync.dma_start(out=wsd0[:], in_=t_wsd[0:128, :])
            nc.sync.dma_start(out=wsd1[:], in_=t_wsd[128:256, :])

            # expand compact index tables to 128 partitions (8x replication)
            ihx = cpool.tile([128, NB * (NIDX // 16)], i16, tag="ihx")
            isx = cpool.tile([128, NB * (NIDX // 16)], i16, tag="isx")
            for k in range(8):
                nc.sync.dma_start(out=ihx[16 * k : 16 * k + 16, :], in_=t_idxh[:, :])
                nc.sync.dma_start(out=isx[16 * k : 16 * k + 16, :], in_=t_idxs[:, :])
            dl16 = cpool.tile([128, NB * B], bf, tag="dl16")
            nc.sync.dma_start(out=dl16[:], in_=t_dstl[:])
            dlf = cpool.tile([128, NB * B], f32, tag="dlf")
            nc.vector.tensor_copy(out=dlf[:], in_=dl16[:])

            # ---------------- phase 1: h_ext = [x@W | a_src | a_dst | 1] ----
            with tc.tile_pool(name="ptp", bufs=2, space="PSUM") as ptp:
                for t in range(NT):
                    n0 = t * P
                    nr = min(P, N - n0)
                    xn = xa.tile([P, IN], bf, tag="xn")
                    nc.sync.dma_start(out=xn[:nr, :], in_=x_full[n0 : n0 + nr, :])
                    pt_ = ptp.tile([P, 2 * P], bf, space="PSUM")
                    nc.tensor.transpose(pt_[:, 0:nr], xn[:nr, 0:P], ident_t[:nr, :nr])
                    nc.tensor.transpose(pt_[:, P : P + nr], xn[:nr, P : 2 * P], ident_t[:nr, :nr])
                    xt = xa.tile([P, 2 * P], bf, tag="xt")
                    nc.vector.tensor_copy(out=xt[:, 0:nr], in_=pt_[:, 0:nr])
                    nc.vector.tensor_copy(out=xt[:, P : P + nr], in_=pt_[:, P : P + nr])
                    ph_t = ph.tile([P, C], f32, space="PSUM")
                    nc.tensor.matmul(out=ph_t[:nr, :], lhsT=xt[:, 0:nr], rhs=w0[:], start=True, stop=False)
                    nc.tensor.matmul(out=ph_t[:nr, :], lhsT=xt[:, P : P + nr], rhs=w1[:], start=False, stop=True)
                    pa_t = pa.tile([P, 2], f32, space="PSUM", tag="pa_t")
                    nc.tensor.matmul(out=pa_t[:nr, :], lhsT=xt[:, 0:nr], rhs=wsd0[:], start=True, stop=False)
                    nc.tensor.matmul(out=pa_t[:nr, :], lhsT=xt[:, P : P + nr], rhs=wsd1[:], start=False, stop=True)
                    h_sb = hs.tile([P, ROW], bf, tag="hsb")
                    nc.vector.tensor_copy(out=h_sb[:nr, 0:C], in_=ph_t[:nr, :])
                    nc.vector.tensor_copy(out=h_sb[:nr, SCOFF : SCOFF + 2], in_=pa_t[:nr, :])
                    nc.vector.memset(h_sb[:nr, SCOFF + 2 : SCOFF + 3], 1.0)
                    nc.sync.dma_start(out=h_ext[n0 : n0 + nr, :], in_=h_sb[:nr, :])
                    sc_sb = hs.tile([P, 128], bf, tag="scsb")
                    nc.vector.tensor_copy(out=sc_sb[:nr, 0:2], in_=pa_t[:nr, :])
                    nc.sync.dma_start(out=sc_tab[n0 : n0 + nr, :], in_=sc_sb[:nr, :])

            tc.strict_bb_all_engine_barrier()

            # ---------------- phase 2: gather / softmax / scatter -----------
            with (
                tc.tile_pool(name="gb", bufs=3) as gb,
                tc.tile_pool(name="scp", bufs=4) as scp,
                tc.tile_pool(name="ohp", bufs=4) as ohp,
                tc.tile_pool(name="po", bufs=4, space="PSUM") as po,
                tc.tile_pool(name="ou", bufs=3) as ou,
            ):
                g_tiles = {}
                e_tiles = {}
                for ev in pp["events"]:
                    if ev[0] == "batch":
                        bi = ev[1]
                        shi, dhi = pp["batches"][bi]
                        c0 = bi * (NIDX // 16)
                        g_t = gb.tile([P, B * ROW], bf, tag="g")
                        s_t = gb.tile([P, B * 128], bf, tag="s")
                        tab = h_ext[HI_OFF:, :] if shi else h_ext[:, :]
                        stab = sc_tab[HI_OFF:, :] if dhi else sc_tab[:, :]
                        QN = 1024
                        for q in range(NIDX // QN):
                            qsl = slice(c0 + q * (QN // 16), c0 + (q + 1) * (QN // 16))
                            gsl = slice(q * (QN // P) * ROW, (q + 1) * (QN // P) * ROW)
                            ssl = slice(q * (QN // P) * 128, (q + 1) * (QN // P) * 128)
                            nc.gpsimd.dma_gather(
                                g_t[:, gsl].rearrange("p (c e) -> p c e", e=ROW),
                                tab, ihx[:, qsl], QN, QN, ROW,
                                single_packet=True,
                            )
                            nc.gpsimd.dma_gather(
                                s_t[:, ssl].rearrange("p (c e) -> p c e", e=128),
                                stab, isx[:, qsl], QN, QN, 128,
                                single_packet=True,
                            )
                        g3 = g_t[:].rearrange("p (c e) -> p c e", e=ROW)
                        s3 = s_t[:].rearrange("p (c e) -> p c e", e=128)
                        ss = scp.tile([P, B], f32, tag="ss")
                        se = scp.tile([P, B], f32, tag="se")
                        nc.vector.tensor_tensor(
                            out=ss[:].rearrange("p (c e) -> p c e", e=1),
                            in0=g3[:, :, SCOFF : SCOFF + 1],
                            in1=s3[:, :, 1:2],
                            op=mybir.AluOpType.add,
                        )
                        nc.scalar.activation(out=ss[:], in_=ss[:], func=mybir.ActivationFunctionType.Prelu, alpha=NEG_SLOPE)
                        nc.scalar.activation(out=se[:], in_=ss[:], func=mybir.ActivationFunctionType.Exp)
                        g_tiles[bi] = g_t
                        e_tiles[bi] = se
                    else:
                        _, t, nr, slots = ev
                        pt = po.tile([P, C + 3], f32, space="PSUM")
                        nch = len(slots)
                        for j, (bi, s) in enumerate(slots):
                            oh_t = ohp.tile([P, P], bf, tag="oh")
                            nc.vector.tensor_scalar(
                                out=oh_t[:],
                                in0=iota_t[:],
                                scalar1=dlf[:, bi * B + s : bi * B + s + 1],
                                scalar2=e_tiles[bi][:, s : s + 1],
                                op0=mybir.AluOpType.is_equal,
                                op1=mybir.AluOpType.mult,
                            )
                            nc.tensor.matmul(
                                out=pt[:, :],
                                lhsT=oh_t[:],
                                rhs=g_tiles[bi][:, s * ROW : s * ROW + C + 3],
                                start=(j == 0),
                                stop=(j == nch - 1),
                            )
                        dn = ou.tile([P, 1], f32, tag="dn")
                        nc.vector.tensor_scalar(
                            out=dn[:], in0=pt[:, C + 2 : C + 3],
                            scalar1=float(H), scalar2=float(H) * EPS,
                            op0=mybir.AluOpType.mult, op1=mybir.AluOpType.add,
                        )
                        rc = ou.tile([P, 1], f32, tag="rc")
                        nc.vector.reciprocal(out=rc[:], in_=dn[:])
                        ob = ou.tile([P, C], f32, tag="ob")
                        nc.vector.tensor_scalar_mul(out=ob[:], in0=pt[:, 0:C], scalar1=rc[:, :1])
                        nc.sync.dma_start(out=rs_in[t * P : t * P + nr, :], in_=ob[:nr, :])

                # ------------ phase 3: ReduceScatter + f16 readback ---------
                tc.strict_bb_all_engine_barrier()
                nc.gpsimd.collective_compute(
                    "ReduceScatter", mybir.AluOpType.add,
                    replica_groups=GRP, ins=[rs_in.opt()], outs=[rs_out.opt()],
                )
                for tt in range(NTS):
                    r0 = tt * P
                    rr = min(P, NS - r0)
                    fb = ou.tile([P, C], f32, tag="fb")
                    nc.sync.dma_start(out=fb[:rr, :], in_=rs_out[r0 : r0 + rr, :])
                    fb16 = ou.tile([P, C], f16, tag="fb16")
                    nc.vector.tensor_copy(out=fb16[:rr, :], in_=fb[:rr, :])
                    nc.sync.dma_start(out=t_out[r0 : r0 + rr, :], in_=fb16[:rr, :])

    nc.finalize()
    return nc


_IOTA = np.broadcast_to(np.arange(P, dtype=np.float32), (P, P)).copy()
_IDENT = np.eye(P, dtype=BF16)

_CACHE = {}


def _get_compiled(edge_index):
    ck = _CACHE.get("edge_index")
    if ck is None or not np.array_equal(ck, edge_index):
        pp = _preprocess(edge_index)
        nc = _build_program(pp)
        _CACHE.update(edge_index=edge_index.copy(), pp=pp, nc=nc, in_key=None)
    return _CACHE["pp"], _CACHE["nc"]


def _make_in_maps(x, W, att_src, att_dst, pp):
    key = _CACHE.get("in_key")
    if key is not None:
        ox, oW, osrc, odst = key
        if (
            np.array_equal(ox, x)
            and np.array_equal(oW, W)
            and np.array_equal(osrc, att_src)
            and np.array_equal(odst, att_dst)
        ):
            return _CACHE["in_maps"]

    xbf = x.astype(BF16)
    base = {
        "iota": _IOTA,
        "ident": _IDENT,
        "idxh": pp["idxh"],
        "idxs": pp["idxs"],
        "dstl": pp["dstl"],
    }
    in_maps = []
    for h in range(H):
        Wh = W[:, h * C : (h + 1) * C].astype(np.float32)
        wsrc = Wh @ att_src[h].astype(np.float32)
        wdst = Wh @ att_dst[h].astype(np.float32)
        m = dict(base)
        m["xsl"] = xbf[h * NS : (h + 1) * NS, :]
        m["Wh"] = Wh.astype(BF16)
        m["wsd"] = np.stack([wsrc, wdst], axis=1).astype(BF16)
        in_maps.append(m)
    _CACHE["in_key"] = (x.copy(), W.copy(), att_src.copy(), att_dst.copy())
    _CACHE["in_maps"] = in_maps
    return in_maps


def kernel(x, edge_index, W, att_src, att_dst, bias, _timing=None):
    x = np.asarray(x)
    edge_index = np.asarray(edge_index)
    W = np.asarray(W)
    att_src = np.asarray(att_src)
    att_dst = np.asarray(att_dst)
    bias = np.asarray(bias)

    pp, nc = _get_compiled(edge_index)
    in_maps = _make_in_maps(x, W, att_src, att_dst, pp)
    res = run_bass_kernel_spmd(nc, in_maps, core_ids=list(range(H)))
    if _timing is not None:
        _timing["exec_time_ns"] = res.exec_time_ns
    out = np.concatenate(
        [res.results[h]["out"] for h in range(H)], axis=0
    ).astype(np.float32)
    out += bias.astype(np.float32)[None, :]
    return out


# revision 9
# speedup vs baseline: 12.6170x; 1.5411x over previous
"""GAT layer (PyG-style, concat=False) on 8 Trainium2 NeuronCores.

Sharding: one attention head per core (H == n_cores == 8), with all large
host<->device traffic minimized (the axon tunnel runs at ~35 MB/s, so wire
bytes dominate wall time):
  - x is sent SHARDED: core c gets rows [c*6250, (c+1)*6250) as bf16 and the
    full [N, IN] table is rebuilt on-device with an AllGather collective.
  - edge index tables are sent compact ([16, .] int16, not replicated to 128
    partitions; dst-locals as bf16) and expanded on-device.
  - each core computes its head's output, scales by 1/8, and a
    ReduceScatter(add) leaves each core with a [6250, C] shard of the final
    head-mean; only that shard (float16) is returned to the host.

Per-core program:
  phase 0: AllGather x shards -> x_full [N, IN] bf16 (internal DRAM).
  phase 1: per 128-node tile: load x rows, PE-transpose, h = x @ W_head
           (bf16 PE matmul), a_src/a_dst matvecs; writes a 768B-per-node
           table h_ext[N, 384] = [h(256)|a_src|a_dst|1.0|pad] + score table.
  phase 2: edges grouped by 128-row dst tiles; per 128-edge chunk, dma_gather
           fetches src rows and dst score rows, scores go through
           Prelu(0.2)+Exp, a fused DVE op builds the exp-scaled one-hot, and
           one PE matmul scatter-accumulates messages + denominator into
           PSUM. Per tile: multiply by 1/(8*(denom+eps)), DMA to rs_in.
  phase 3: ReduceScatter(add) rs_in -> [6250, C] shard; cast f16; DMA out.
Host concatenates the 8 shards and adds bias.
"""

import numpy as np
import ml_dtypes

try:  # persistent XLA compile cache cuts repeat-call jit overhead
    import jax
    jax.config.update("jax_compilation_cache_dir", "/tmp/jax_cache")
    jax.config.update("jax_persistent_cache_min_entry_size_bytes", -1)
    jax.config.update("jax_persistent_cache_min_compile_time_secs", 0)
except Exception:
    pass

import concourse.bass as bass
import concourse.bacc as bacc
import concourse.mybir as mybir
from concourse.tile import TileContext
from concourse.bass_utils import run_bass_kernel_spmd

N = 50000
E = 200000
H = 8
C = 256
IN = 256
NEG_SLOPE = 0.2
EPS = 1e-16

P = 128
NT = (N + P - 1) // P            # 391 dst tiles (last has 80 rows)
NS = N // H                      # 6250 output rows per core
NTS = (NS + P - 1) // P          # 49 readback tiles (last has 106 rows)
ROW = 384                        # h_ext row width (bf16) -> 768B
SCOFF = 256                      # score columns start (a_src, a_dst, one)
B = 32                           # chunks per gather batch
NIDX = B * P                     # indices per batch (4096)
HI_OFF = 17232                   # high-table row offset (N-1-HI_OFF <= 32767)
BF16 = ml_dtypes.bfloat16
FP8 = ml_dtypes.float8_e4m3
X_FP8 = False                    # fp8 x fails the 2e-2 rel-err budget (2.8e-2)


def _wrap16(ix):
    """[NIDX] int -> [16, NIDX//16] int16 (16-partition wrapped, compact)."""
    return ix.reshape(-1, 16).T.astype(np.int16)


def _preprocess(edge_index):
    """Build chunk/batch structures shared by all cores.

    Returns dict with:
      idxh  [16, NB*NIDX//16] int16   row-gather indices per batch (wrapped)
      idxs  [16, NB*NIDX//16] int16   score-gather indices per batch (wrapped)
      dstl  [128, NB*B] bf16          local dst per chunk slot (-1 = pad)
      batches: list of (src_hi, dst_hi)
      events: list of ('batch', b) / ('tile', t, nr, [(b, slot), ...])
    """
    src = edge_index[0].astype(np.int64)
    dst = edge_index[1].astype(np.int64)
    order = np.argsort(dst, kind="stable")
    dst_sorted = dst[order]
    tile_starts = np.searchsorted(dst_sorted, np.arange(0, NT * P + 1, P))

    # --- build chunks per tile (tile-major order) ---
    chunks = []
    tile_chunk_ids = [[] for _ in range(NT)]
    for t in range(NT):
        lo_, hi_ = tile_starts[t], tile_starts[t + 1]
        eids = order[lo_:hi_]
        if len(eids):
            eids = eids[np.argsort(src[eids], kind="stable")]
            s = src[eids]
            cut = int(np.searchsorted(s, 32768))
            parts = [(eids[:cut], False), (eids[cut:], True)]
        else:
            parts = [(eids, False)]  # ensure >=1 chunk to zero the PSUM
        got = False
        for part, shi in parts:
            if len(part) == 0 and got:
                continue
            if len(part) == 0:
                tile_chunk_ids[t].append(len(chunks))
                chunks.append((t, part, shi))
                got = True
                continue
            for i in range(0, len(part), P):
                tile_chunk_ids[t].append(len(chunks))
                chunks.append((t, part[i : i + P], shi))
                got = True

    # --- assign chunks to class-pure batches of B, emit events ---
    batches = []        # (src_hi, dst_hi)
    batch_slots = []    # list per batch: list of chunk ids (or -1 pad)
    open_batches = {}   # (src_hi, dst_hi) -> batch idx
    chunk_pos = {}      # chunk id -> (batch, slot)
    closed = set()
    events = []
    tiles_pending = []
    emitted_tiles = set()

    def close_batch(bi):
        while len(batch_slots[bi]) < B:
            batch_slots[bi].append(-1)
        closed.add(bi)
        events.append(("batch", bi))
        still = []
        for t in tiles_pending:
            if all(chunk_pos[c][0] in closed for c in tile_chunk_ids[t]):
                nr = min(P, N - t * P)
                events.append(
                    ("tile", t, nr, [chunk_pos[c] for c in tile_chunk_ids[t]])
                )
                emitted_tiles.add(t)
            else:
                still.append(t)
        tiles_pending[:] = still

    cur_dst_hi = False
    for t in range(NT):
        dst_hi = t >= 256
        if dst_hi and not cur_dst_hi:
            for key in list(open_batches):
                close_batch(open_batches.pop(key))
            cur_dst_hi = True
        for c in tile_chunk_ids[t]:
            _, _, shi = chunks[c]
            key = (shi, dst_hi)
            if key not in open_batches:
                batches.append(key)
                batch_slots.append([])
                open_batches[key] = len(batches) - 1
            bi = open_batches[key]
            chunk_pos[c] = (bi, len(batch_slots[bi]))
            batch_slots[bi].append(c)
            if len(batch_slots[bi]) == B:
                del open_batches[key]
                close_batch(bi)
        tiles_pending.append(t)
    for key in list(open_batches):
        close_batch(open_batches.pop(key))
    assert not tiles_pending and len(emitted_tiles) == NT

    # --- build compact index arrays ---
    NB = len(batches)
    idxh = np.zeros((16, NB * (NIDX // 16)), np.int16)
    idxs = np.zeros((16, NB * (NIDX // 16)), np.int16)
    dstl = np.full((128, NB * B), -1.0, BF16)
    for bi, (shi, dhi) in enumerate(batches):
        hix = np.zeros(NIDX, np.int64)
        six = np.zeros(NIDX, np.int64)
        for s_i, c in enumerate(batch_slots[bi]):
            if c < 0:
                continue
            t, eids, c_shi = chunks[c]
            ne = len(eids)
            if ne:
                sv = src[eids] - (HI_OFF if c_shi else 0)
                dv = dst[eids] - (HI_OFF if dhi else 0)
                hix[s_i * P : s_i * P + ne] = sv
                six[s_i * P : s_i * P + ne] = dv
                dstl[:ne, bi * B + s_i] = (dst[eids] - t * P).astype(BF16)
        idxh[:, bi * (NIDX // 16) : (bi + 1) * (NIDX // 16)] = _wrap16(hix)
        idxs[:, bi * (NIDX // 16) : (bi + 1) * (NIDX // 16)] = _wrap16(six)

    return {
        "idxh": idxh,
        "idxs": idxs,
        "dstl": dstl,
        "batches": batches,
        "events": events,
    }


def _build_program(pp):
    """Build the per-core Bacc program (identical for all cores)."""
    NB = len(pp["batches"])
    KB = (NB + 7) // 8               # batch-blocks per core (idx sharding)
    NB8 = KB * 8
    QW = NIDX // 16                  # idx columns per batch (256)
    nc = bacc.Bacc(num_devices=8)
    bf = mybir.dt.bfloat16
    f16 = mybir.dt.float16
    f32 = mybir.dt.float32
    i16 = mybir.dt.int16
    xdt = mybir.dt.float8e4 if X_FP8 else bf
    GRP = [list(range(8))]

    t_xsl = nc.declare_dram_parameter("xsl", [NS, IN], xdt, isOutput=False)
    t_W = nc.declare_dram_parameter("Wh", [IN, C], bf, isOutput=False)
    t_wsd = nc.declare_dram_parameter("wsd", [IN, 2], bf, isOutput=False)
    t_iota = nc.declare_dram_parameter("iota", [P, P], f32, isOutput=False)
    t_ident = nc.declare_dram_parameter("ident", [P, P], bf, isOutput=False)
    t_idxh = nc.declare_dram_parameter("idxh", [16, KB * QW], i16, isOutput=False)
    t_idxs = nc.declare_dram_parameter("idxs", [16, KB * QW], i16, isOutput=False)
    t_dstl = nc.declare_dram_parameter("dstl", [128, KB * B], bf, isOutput=False)
    t_out = nc.declare_dram_parameter("out", [NS, C], f16, isOutput=True)

    h_ext = nc.dram_tensor("h_ext", [N, ROW], bf)
    sc_tab = nc.dram_tensor("sc_tab", [N, 128], bf)

    with TileContext(nc) as tc:
        with (
            tc.tile_pool(name="dramp", bufs=1, space="DRAM") as dramp,
            tc.tile_pool(name="const", bufs=1) as cpool,
            tc.tile_pool(name="xa", bufs=4) as xa,
            tc.tile_pool(name="hs", bufs=3) as hs,
            tc.tile_pool(name="ph", bufs=2, space="PSUM") as ph,
            tc.tile_pool(name="pa", bufs=2, space="PSUM") as pa,
        ):
            x_bounce = dramp.tile([NS, IN], xdt)
            x_full = dramp.tile([N, IN], xdt)
            ih_b = dramp.tile([16, KB * QW], i16)
            ih_g = dramp.tile([128, KB * QW], i16)
            is_b = dramp.tile([16, KB * QW], i16)
            is_g = dramp.tile([128, KB * QW], i16)
            dl_b = dramp.tile([128, KB * B], bf)
            dl_g = dramp.tile([1024, KB * B], bf)
            rs_in = dramp.tile([N, C], f32)
            rs_out = dramp.tile([NS, C], f32)

            # ------------- phase 0: AllGather x + idx-table shards ----------
            nc.gpsimd.dma_start(x_bounce[:], t_xsl[:])
            nc.gpsimd.collective_compute(
                "AllGather", mybir.AluOpType.bypass,
                replica_groups=GRP, ins=[x_bounce.opt()], outs=[x_full.opt()],
            )
            nc.gpsimd.dma_start(ih_b[:], t_idxh[:])
            nc.gpsimd.collective_compute(
                "AllGather", mybir.AluOpType.bypass,
                replica_groups=GRP, ins=[ih_b.opt()], outs=[ih_g.opt()],
            )
            nc.gpsimd.dma_start(is_b[:], t_idxs[:])
            nc.gpsimd.collective_compute(
                "AllGather", mybir.AluOpType.bypass,
                replica_groups=GRP, ins=[is_b.opt()], outs=[is_g.opt()],
            )
            nc.gpsimd.dma_start(dl_b[:], t_dstl[:])
            nc.gpsimd.collective_compute(
                "AllGather", mybir.AluOpType.bypass,
                replica_groups=GRP, ins=[dl_b.opt()], outs=[dl_g.opt()],
            )

            iota_t = cpool.tile([P, P], f32)
            nc.sync.dma_start(out=iota_t[:], in_=t_iota[:])
            ident_t = cpool.tile([P, P], bf)
            nc.sync.dma_start(out=ident_t[:], in_=t_ident[:])
            w0 = cpool.tile([128, C], bf, tag="w0")
            w1 = cpool.tile([128, C], bf, tag="w1")
            nc.sync.dma_start(out=w0[:], in_=t_W[0:128, :])
            nc.sync.dma_start(out=w1[:], in_=t_W[128:256, :])
            wsd0 = cpool.tile([128, 2], bf, tag="wsd0")
            wsd1 = cpool.tile([128, 2], bf, tag="wsd1")
            nc.sync.dma_start(out=wsd0[:], in_=t_wsd[0:128, :])
            nc.sync.dma_start(out=wsd1[:], in_=t_wsd[128:256, :])

            # expand gathered idx tables to the 128-partition SBUF layout
            # (8x partition replication; batch-block b holds batches
            #  [b*KB, (b+1)*KB) of the global order)
            ihx = cpool.tile([128, NB8 * QW], i16, tag="ihx")
            isx = cpool.tile([128, NB8 * QW], i16, tag="isx")
            for k in range(8):
                for b in range(8):
                    csl = slice(b * KB * QW, (b + 1) * KB * QW)
                    nc.sync.dma_start(out=ihx[16 * k : 16 * k + 16, csl], in_=ih_g[16 * b : 16 * b + 16, :])
                    nc.sync.dma_start(out=isx[16 * k : 16 * k + 16, csl], in_=is_g[16 * b : 16 * b + 16, :])
            dl16 = cpool.tile([128, NB8 * B], bf, tag="dl16")
            for b in range(8):
                nc.sync.dma_start(out=dl16[:, b * KB * B : (b + 1) * KB * B], in_=dl_g[128 * b : 128 * (b + 1), :])
            dlf = cpool.tile([128, NB8 * B], f32, tag="dlf")
            nc.vector.tensor_copy(out=dlf[:], in_=dl16[:])

            # ---------------- phase 1: h_ext = [x@W | a_src | a_dst | 1] ----
            with tc.tile_pool(name="ptp", bufs=2, space="PSUM") as ptp:
                for t in range(NT):
                    n0 = t * P
                    nr = min(P, N - n0)
                    if X_FP8:
                        xn8 = xa.tile([P, IN], mybir.dt.float8e4, tag="xn8")
                        nc.sync.dma_start(out=xn8[:nr, :], in_=x_full[n0 : n0 + nr, :])
                        xn = xa.tile([P, IN], bf, tag="xn")
                        nc.vector.tensor_copy(out=xn[:nr, :], in_=xn8[:nr, :])
                    else:
                        xn = xa.tile([P, IN], bf, tag="xn")
                        nc.sync.dma_start(out=xn[:nr, :], in_=x_full[n0 : n0 + nr, :])
                    pt_ = ptp.tile([P, 2 * P], bf, space="PSUM")
                    nc.tensor.transpose(pt_[:, 0:nr], xn[:nr, 0:P], ident_t[:nr, :nr])
                    nc.tensor.transpose(pt_[:, P : P + nr], xn[:nr, P : 2 * P], ident_t[:nr, :nr])
                    xt = xa.tile([P, 2 * P], bf, tag="xt")
                    nc.vector.tensor_copy(out=xt[:, 0:nr], in_=pt_[:, 0:nr])
                    nc.vector.tensor_copy(out=xt[:, P : P + nr], in_=pt_[:, P : P + nr])
                    ph_t = ph.tile([P, C], f32, space="PSUM")
                    nc.tensor.matmul(out=ph_t[:nr, :], lhsT=xt[:, 0:nr], rhs=w0[:], start=True, stop=False)
                    nc.tensor.matmul(out=ph_t[:nr, :], lhsT=xt[:, P : P + nr], rhs=w1[:], start=False, stop=True)
                    pa_t = pa.tile([P, 2], f32, space="PSUM", tag="pa_t")
                    nc.tensor.matmul(out=pa_t[:nr, :], lhsT=xt[:, 0:nr], rhs=wsd0[:], start=True, stop=False)
                    nc.tensor.matmul(out=pa_t[:nr, :], lhsT=xt[:, P : P + nr], rhs=wsd1[:], start=False, stop=True)
                    h_sb = hs.tile([P, ROW], bf, tag="hsb")
                    nc.vector.tensor_copy(out=h_sb[:nr, 0:C], in_=ph_t[:nr, :])
                    nc.vector.tensor_copy(out=h_sb[:nr, SCOFF : SCOFF + 2], in_=pa_t[:nr, :])
                    nc.vector.memset(h_sb[:nr, SCOFF + 2 : SCOFF + 3], 1.0)
                    nc.sync.dma_start(out=h_ext[n0 : n0 + nr, :], in_=h_sb[:nr, :])
                    sc_sb = hs.tile([P, 128], bf, tag="scsb")
                    nc.vector.tensor_copy(out=sc_sb[:nr, 0:2], in_=pa_t[:nr, :])
                    nc.sync.dma_start(out=sc_tab[n0 : n0 + nr, :], in_=sc_sb[:nr, :])

            tc.strict_bb_all_engine_barrier()

            # ---------------- phase 2: gather / softmax / scatter -----------
            with (
                tc.tile_pool(name="gb", bufs=3) as gb,
                tc.tile_pool(name="scp", bufs=4) as scp,
                tc.tile_pool(name="ohp", bufs=4) as ohp,
                tc.tile_pool(name="po", bufs=4, space="PSUM") as po,
                tc.tile_pool(name="ou", bufs=3) as ou,
            ):
                g_tiles = {}
                e_tiles = {}
                for ev in pp["events"]:
                    if ev[0] == "batch":
                        bi = ev[1]
                        shi, dhi = pp["batches"][bi]
                        c0 = bi * (NIDX // 16)
                        g_t = gb.tile([P, B * ROW], bf, tag="g")
                        s_t = gb.tile([P, B * 128], bf, tag="s")
                        tab = h_ext[HI_OFF:, :] if shi else h_ext[:, :]
                        stab = sc_tab[HI_OFF:, :] if dhi else sc_tab[:, :]
                        QN = 1024
                        for q in range(NIDX // QN):
                            qsl = slice(c0 + q * (QN // 16), c0 + (q + 1) * (QN // 16))
                            gsl = slice(q * (QN // P) * ROW, (q + 1) * (QN // P) * ROW)
                            ssl = slice(q * (QN // P) * 128, (q + 1) * (QN // P) * 128)
                            nc.gpsimd.dma_gather(
                                g_t[:, gsl].rearrange("p (c e) -> p c e", e=ROW),
                                tab, ihx[:, qsl], QN, QN, ROW,
                                single_packet=True,
                            )
                            nc.gpsimd.dma_gather(
                                s_t[:, ssl].rearrange("p (c e) -> p c e", e=128),
                                stab, isx[:, qsl], QN, QN, 128,
                                single_packet=True,
                            )
                        g3 = g_t[:].rearrange("p (c e) -> p c e", e=ROW)
                        s3 = s_t[:].rearrange("p (c e) -> p c e", e=128)
                        ss = scp.tile([P, B], f32, tag="ss")
                        se = scp.tile([P, B], f32, tag="se")
                        nc.vector.tensor_tensor(
                            out=ss[:].rearrange("p (c e) -> p c e", e=1),
                            in0=g3[:, :, SCOFF : SCOFF + 1],
                            in1=s3[:, :, 1:2],
                            op=mybir.AluOpType.add,
                        )
                        nc.scalar.activation(out=ss[:], in_=ss[:], func=mybir.ActivationFunctionType.Prelu, alpha=NEG_SLOPE)
                        nc.scalar.activation(out=se[:], in_=ss[:], func=mybir.ActivationFunctionType.Exp)
                        g_tiles[bi] = g_t
                        e_tiles[bi] = se
                    else:
                        _, t, nr, slots = ev
                        pt = po.tile([P, C + 3], f32, space="PSUM")
                        nch = len(slots)
                        for j, (bi, s) in enumerate(slots):
                            oh_t = ohp.tile([P, P], bf, tag="oh")
                            nc.vector.tensor_scalar(
                                out=oh_t[:],
                                in0=iota_t[:],
                                scalar1=dlf[:, bi * B + s : bi * B + s + 1],
                                scalar2=e_tiles[bi][:, s : s + 1],
                                op0=mybir.AluOpType.is_equal,
                                op1=mybir.AluOpType.mult,
                            )
                            nc.tensor.matmul(
                                out=pt[:, :],
                                lhsT=oh_t[:],
                                rhs=g_tiles[bi][:, s * ROW : s * ROW + C + 3],
                                start=(j == 0),
                                stop=(j == nch - 1),
                            )
                        dn = ou.tile([P, 1], f32, tag="dn")
                        nc.vector.tensor_scalar(
                            out=dn[:], in0=pt[:, C + 2 : C + 3],
                            scalar1=float(H), scalar2=float(H) * EPS,
                            op0=mybir.AluOpType.mult, op1=mybir.AluOpType.add,
                        )
                        rc = ou.tile([P, 1], f32, tag="rc")
                        nc.vector.reciprocal(out=rc[:], in_=dn[:])
                        ob = ou.tile([P, C], f32, tag="ob")
                        nc.vector.tensor_scalar_mul(out=ob[:], in0=pt[:, 0:C], scalar1=rc[:, :1])
                        nc.sync.dma_start(out=rs_in[t * P : t * P + nr, :], in_=ob[:nr, :])

                # ------------ phase 3: ReduceScatter + f16 readback ---------
                tc.strict_bb_all_engine_barrier()
                nc.gpsimd.collective_compute(
                    "ReduceScatter", mybir.AluOpType.add,
                    replica_groups=GRP, ins=[rs_in.opt()], outs=[rs_out.opt()],
                )
                for tt in range(NTS):
                    r0 = tt * P
                    rr = min(P, NS - r0)
                    fb = ou.tile([P, C], f32, tag="fb")
                    nc.sync.dma_start(out=fb[:rr, :], in_=rs_out[r0 : r0 + rr, :])
                    fb16 = ou.tile([P, C], f16, tag="fb16")
                    nc.vector.tensor_copy(out=fb16[:rr, :], in_=fb[:rr, :])
                    nc.sync.dma_start(out=t_out[r0 : r0 + rr, :], in_=fb16[:rr, :])

    nc.finalize()
    return nc


_IOTA = np.broadcast_to(np.arange(P, dtype=np.float32), (P, P)).copy()
_IDENT = np.eye(P, dtype=BF16)

_CACHE = {}


def _get_compiled(edge_index):
    ck = _CACHE.get("edge_index")
    if ck is None or not np.array_equal(ck, edge_index):
        pp = _preprocess(edge_index)
        nc = _build_program(pp)
        _CACHE.update(edge_index=edge_index.copy(), pp=pp, nc=nc, in_key=None)
    return _CACHE["pp"], _CACHE["nc"]


def _make_in_maps(x, W, att_src, att_dst, pp):
    key = _CACHE.get("in_key")
    if key is not None:
        ox, oW, osrc, odst = key
        if (
            np.array_equal(ox, x)
            and np.array_equal(oW, W)
            and np.array_equal(osrc, att_src)
            and np.array_equal(odst, att_dst)
        ):
            return _CACHE["in_maps"]

    xq = x.astype(FP8 if X_FP8 else BF16)
    NB = len(pp["batches"])
    KB = (NB + 7) // 8
    NB8 = KB * 8
    QW = NIDX // 16
    idxh = np.zeros((16, NB8 * QW), np.int16)
    idxh[:, : NB * QW] = pp["idxh"]
    idxs = np.zeros((16, NB8 * QW), np.int16)
    idxs[:, : NB * QW] = pp["idxs"]
    dstl = np.full((128, NB8 * B), -1.0, BF16)
    dstl[:, : NB * B] = pp["dstl"]
    base = {"iota": _IOTA, "ident": _IDENT}
    in_maps = []
    for h in range(H):
        Wh = W[:, h * C : (h + 1) * C].astype(np.float32)
        wsrc = Wh @ att_src[h].astype(np.float32)
        wdst = Wh @ att_dst[h].astype(np.float32)
        m = dict(base)
        m["xsl"] = xq[h * NS : (h + 1) * NS, :]
        m["Wh"] = Wh.astype(BF16)
        m["wsd"] = np.stack([wsrc, wdst], axis=1).astype(BF16)
        m["idxh"] = idxh[:, h * KB * QW : (h + 1) * KB * QW]
        m["idxs"] = idxs[:, h * KB * QW : (h + 1) * KB * QW]
        m["dstl"] = dstl[:, h * KB * B : (h + 1) * KB * B]
        in_maps.append(m)
    _CACHE["in_key"] = (x.copy(), W.copy(), att_src.copy(), att_dst.copy())
    _CACHE["in_maps"] = in_maps
    return in_maps


def kernel(x, edge_index, W, att_src, att_dst, bias, _timing=None):
    x = np.asarray(x)
    edge_index = np.asarray(edge_index)
    W = np.asarray(W)
    att_src = np.asarray(att_src)
    att_dst = np.asarray(att_dst)
    bias = np.asarray(bias)

    pp, nc = _get_compiled(edge_index)
    in_maps = _make_in_maps(x, W, att_src, att_dst, pp)
    res = run_bass_kernel_spmd(nc, in_maps, core_ids=list(range(H)))
    if _timing is not None:
        _timing["exec_time_ns"] = res.exec_time_ns
    out = np.concatenate(
        [res.results[h]["out"] for h in range(H)], axis=0
    ).astype(np.float32)
    out += bias.astype(np.float32)[None, :]
    return out


# revision 15
# speedup vs baseline: 14.1917x; 1.1248x over previous
"""GAT layer (PyG-style, concat=False) on 8 Trainium2 NeuronCores.

Sharding: one attention head per core (H == n_cores == 8), with all large
host<->device traffic minimized (the axon tunnel runs at ~35 MB/s, so wire
bytes dominate wall time):
  - x is sent SHARDED: core c gets rows [c*6250, (c+1)*6250) as bf16 and the
    full [N, IN] table is rebuilt on-device with an AllGather collective.
  - edge index tables are sent compact ([16, .] int16, not replicated to 128
    partitions; dst-locals as bf16) and expanded on-device.
  - each core computes its head's output, scales by 1/8, and a
    ReduceScatter(add) leaves each core with a [6250, C] shard of the final
    head-mean; only that shard (float16) is returned to the host.

Per-core program:
  phase 0: AllGather x shards -> x_full [N, IN] bf16 (internal DRAM).
  phase 1: per 128-node tile: load x rows, PE-transpose, h = x @ W_head
           (bf16 PE matmul), a_src/a_dst matvecs; writes a 768B-per-node
           table h_ext[N, 384] = [h(256)|a_src|a_dst|1.0|pad] + score table.
  phase 2: edges grouped by 128-row dst tiles; per 128-edge chunk, dma_gather
           fetches src rows and dst score rows, scores go through
           Prelu(0.2)+Exp, a fused DVE op builds the exp-scaled one-hot, and
           one PE matmul scatter-accumulates messages + denominator into
           PSUM. Per tile: multiply by 1/(8*(denom+eps)), DMA to rs_in.
  phase 3: ReduceScatter(add) rs_in -> [6250, C] shard; cast f16; DMA out.
Host concatenates the 8 shards and adds bias.
"""

import numpy as np
import ml_dtypes

try:  # persistent XLA compile cache cuts repeat-call jit overhead
    import jax
    jax.config.update("jax_compilation_cache_dir", "/tmp/jax_cache")
    jax.config.update("jax_persistent_cache_min_entry_size_bytes", -1)
    jax.config.update("jax_persistent_cache_min_compile_time_secs", 0)
except Exception:
    pass

import concourse.bass as bass
import concourse.bacc as bacc
import concourse.mybir as mybir
from concourse.tile import TileContext
from concourse.bass_utils import run_bass_kernel_spmd

N = 50000
E = 200000
H = 8
C = 256
IN = 256
NEG_SLOPE = 0.2
EPS = 1e-16

P = 128
NT = (N + P - 1) // P            # 391 dst tiles (last has 80 rows)
NS = N // H                      # 6250 output rows per core
NTS = (NS + P - 1) // P          # 49 readback tiles (last has 106 rows)
ROW = 384                        # h_ext row width (bf16) -> 768B
SCOFF = 256                      # score columns start (a_src, a_dst, one)
B = 32                           # chunks per gather batch
NIDX = B * P                     # indices per batch (4096)
HI_OFF = 17232                   # high-table row offset (N-1-HI_OFF <= 32767)
BF16 = ml_dtypes.bfloat16
X_INT8 = True                    # ship x as int8 + per-row absmax scale
                                 # (fp8-e4m3 x fails the 2e-2 budget: 2.8e-2;
                                 #  int8 row-scaled is ~4.5x more precise)


def _wrap16(ix):
    """[NIDX] int -> [16, NIDX//16] int16 (16-partition wrapped, compact)."""
    return ix.reshape(-1, 16).T.astype(np.int16)


def _preprocess(edge_index):
    """Build chunk/batch structures shared by all cores.

    Returns dict with:
      idxh  [16, NB*NIDX//16] int16   row-gather indices per batch (wrapped)
      idxs  [16, NB*NIDX//16] int16   score-gather indices per batch (wrapped)
      dstl  [128, NB*B] bf16          local dst per chunk slot (-1 = pad)
      batches: list of (src_hi, dst_hi)
      events: list of ('batch', b) / ('tile', t, nr, [(b, slot), ...])
    """
    src = edge_index[0].astype(np.int64)
    dst = edge_index[1].astype(np.int64)
    order = np.argsort(dst, kind="stable")
    dst_sorted = dst[order]
    tile_starts = np.searchsorted(dst_sorted, np.arange(0, NT * P + 1, P))

    # --- build chunks per tile (tile-major order) ---
    chunks = []
    tile_chunk_ids = [[] for _ in range(NT)]
    for t in range(NT):
        lo_, hi_ = tile_starts[t], tile_starts[t + 1]
        eids = order[lo_:hi_]
        if len(eids):
            eids = eids[np.argsort(src[eids], kind="stable")]
            s = src[eids]
            cut = int(np.searchsorted(s, 32768))
            parts = [(eids[:cut], False), (eids[cut:], True)]
        else:
            parts = [(eids, False)]  # ensure >=1 chunk to zero the PSUM
        got = False
        for part, shi in parts:
            if len(part) == 0 and got:
                continue
            if len(part) == 0:
                tile_chunk_ids[t].append(len(chunks))
                chunks.append((t, part, shi))
                got = True
                continue
            for i in range(0, len(part), P):
                tile_chunk_ids[t].append(len(chunks))
                chunks.append((t, part[i : i + P], shi))
                got = True

    # --- assign chunks to class-pure batches of B, emit events ---
    batches = []        # (src_hi, dst_hi)
    batch_slots = []    # list per batch: list of chunk ids (or -1 pad)
    open_batches = {}   # (src_hi, dst_hi) -> batch idx
    chunk_pos = {}      # chunk id -> (batch, slot)
    closed = set()
    events = []
    tiles_pending = []
    emitted_tiles = set()

    def close_batch(bi):
        while len(batch_slots[bi]) < B:
            batch_slots[bi].append(-1)
        closed.add(bi)
        events.append(("batch", bi))
        still = []
        for t in tiles_pending:
            if all(chunk_pos[c][0] in closed for c in tile_chunk_ids[t]):
                nr = min(P, N - t * P)
                events.append(
                    ("tile", t, nr, [chunk_pos[c] for c in tile_chunk_ids[t]])
                )
                emitted_tiles.add(t)
            else:
                still.append(t)
        tiles_pending[:] = still

    cur_dst_hi = False
    for t in range(NT):
        dst_hi = t >= 256
        if dst_hi and not cur_dst_hi:
            for key in list(open_batches):
                close_batch(open_batches.pop(key))
            cur_dst_hi = True
        for c in tile_chunk_ids[t]:
            _, _, shi = chunks[c]
            key = (shi, dst_hi)
            if key not in open_batches:
                batches.append(key)
                batch_slots.append([])
                open_batches[key] = len(batches) - 1
            bi = open_batches[key]
            chunk_pos[c] = (bi, len(batch_slots[bi]))
            batch_slots[bi].append(c)
            if len(batch_slots[bi]) == B:
                del open_batches[key]
                close_batch(bi)
        tiles_pending.append(t)
    for key in list(open_batches):
        close_batch(open_batches.pop(key))
    assert not tiles_pending and len(emitted_tiles) == NT

    # --- build compact index arrays ---
    NB = len(batches)
    idxh = np.zeros((16, NB * (NIDX // 16)), np.int16)
    idxs = np.zeros((16, NB * (NIDX // 16)), np.int16)
    dstl = np.full((128, NB * B), -1.0, BF16)
    for bi, (shi, dhi) in enumerate(batches):
        hix = np.zeros(NIDX, np.int64)
        six = np.zeros(NIDX, np.int64)
        for s_i, c in enumerate(batch_slots[bi]):
            if c < 0:
                continue
            t, eids, c_shi = chunks[c]
            ne = len(eids)
            if ne:
                sv = src[eids] - (HI_OFF if c_shi else 0)
                dv = dst[eids] - (HI_OFF if dhi else 0)
                hix[s_i * P : s_i * P + ne] = sv
                six[s_i * P : s_i * P + ne] = dv
                dstl[:ne, bi * B + s_i] = (dst[eids] - t * P).astype(BF16)
        idxh[:, bi * (NIDX // 16) : (bi + 1) * (NIDX // 16)] = _wrap16(hix)
        idxs[:, bi * (NIDX // 16) : (bi + 1) * (NIDX // 16)] = _wrap16(six)

    return {
        "idxh": idxh,
        "idxs": idxs,
        "dstl": dstl,
        "batches": batches,
        "events": events,
    }


def _build_program(pp):
    """Build the per-core Bacc program (identical for all cores)."""
    NB = len(pp["batches"])
    KB = (NB + 7) // 8               # batch-blocks per core (idx sharding)
    NB8 = KB * 8
    QW = NIDX // 16                  # idx columns per batch (256)
    nc = bacc.Bacc(num_devices=8)
    bf = mybir.dt.bfloat16
    f16 = mybir.dt.float16
    f32 = mybir.dt.float32
    i16 = mybir.dt.int16
    xdt = mybir.dt.int8 if X_INT8 else bf
    GRP = [list(range(8))]

    t_xsl = nc.declare_dram_parameter("xsl", [NS, IN], xdt, isOutput=False)
    if X_INT8:
        t_xsc = nc.declare_dram_parameter("xsc", [NS, 1], f32, isOutput=False)
    t_W = nc.declare_dram_parameter("Wh", [IN, C], bf, isOutput=False)
    t_wsd = nc.declare_dram_parameter("wsd", [IN, 2], bf, isOutput=False)
    t_iota = nc.declare_dram_parameter("iota", [P, P], f32, isOutput=False)
    t_ident = nc.declare_dram_parameter("ident", [P, P], bf, isOutput=False)
    t_idxh = nc.declare_dram_parameter("idxh", [16, KB * QW], i16, isOutput=False)
    t_idxs = nc.declare_dram_parameter("idxs", [16, KB * QW], i16, isOutput=False)
    t_dstl = nc.declare_dram_parameter("dstl", [128, KB * B], bf, isOutput=False)
    t_out = nc.declare_dram_parameter("out", [NS, C], f16, isOutput=True)

    h_ext = nc.dram_tensor("h_ext", [N, ROW], bf)
    sc_tab = nc.dram_tensor("sc_tab", [N, 128], bf)

    with TileContext(nc) as tc:
        with (
            tc.tile_pool(name="dramp", bufs=1, space="DRAM") as dramp,
            tc.tile_pool(name="const", bufs=1) as cpool,
            tc.tile_pool(name="xa", bufs=4) as xa,
            tc.tile_pool(name="hs", bufs=3) as hs,
            tc.tile_pool(name="ph", bufs=2, space="PSUM") as ph,
            tc.tile_pool(name="pa", bufs=2, space="PSUM") as pa,
        ):
            x_bounce = dramp.tile([NS, IN], xdt)
            x_full = dramp.tile([N, IN], xdt)
            ih_b = dramp.tile([16, KB * QW], i16)
            ih_g = dramp.tile([128, KB * QW], i16)
            is_b = dramp.tile([16, KB * QW], i16)
            is_g = dramp.tile([128, KB * QW], i16)
            dl_b = dramp.tile([128, KB * B], bf)
            dl_g = dramp.tile([1024, KB * B], bf)
            rs_in = dramp.tile([N, C], f32)
            rs_out = dramp.tile([NS, C], f32)

            # ------------- phase 0: AllGather x + idx-table shards ----------
            nc.gpsimd.dma_start(x_bounce[:], t_xsl[:])
            nc.gpsimd.collective_compute(
                "AllGather", mybir.AluOpType.bypass,
                replica_groups=GRP, ins=[x_bounce.opt()], outs=[x_full.opt()],
            )
            if X_INT8:
                xs_b = dramp.tile([NS, 1], f32)
                xs_g = dramp.tile([N, 1], f32)
                nc.gpsimd.dma_start(xs_b[:], t_xsc[:])
                nc.gpsimd.collective_compute(
                    "AllGather", mybir.AluOpType.bypass,
                    replica_groups=GRP, ins=[xs_b.opt()], outs=[xs_g.opt()],
                )
            nc.gpsimd.dma_start(ih_b[:], t_idxh[:])
            nc.gpsimd.collective_compute(
                "AllGather", mybir.AluOpType.bypass,
                replica_groups=GRP, ins=[ih_b.opt()], outs=[ih_g.opt()],
            )
            nc.gpsimd.dma_start(is_b[:], t_idxs[:])
            nc.gpsimd.collective_compute(
                "AllGather", mybir.AluOpType.bypass,
                replica_groups=GRP, ins=[is_b.opt()], outs=[is_g.opt()],
            )
            nc.gpsimd.dma_start(dl_b[:], t_dstl[:])
            nc.gpsimd.collective_compute(
                "AllGather", mybir.AluOpType.bypass,
                replica_groups=GRP, ins=[dl_b.opt()], outs=[dl_g.opt()],
            )

            iota_t = cpool.tile([P, P], f32)
            nc.sync.dma_start(out=iota_t[:], in_=t_iota[:])
            ident_t = cpool.tile([P, P], bf)
            nc.sync.dma_start(out=ident_t[:], in_=t_ident[:])
            w0 = cpool.tile([128, C], bf, tag="w0")
            w1 = cpool.tile([128, C], bf, tag="w1")
            nc.sync.dma_start(out=w0[:], in_=t_W[0:128, :])
            nc.sync.dma_start(out=w1[:], in_=t_W[128:256, :])
            wsd0 = cpool.tile([128, 2], bf, tag="wsd0")
            wsd1 = cpool.tile([128, 2], bf, tag="wsd1")
            nc.sync.dma_start(out=wsd0[:], in_=t_wsd[0:128, :])
            nc.sync.dma_start(out=wsd1[:], in_=t_wsd[128:256, :])

            # expand gathered idx tables to the 128-partition SBUF layout
            # (8x partition replication; batch-block b holds batches
            #  [b*KB, (b+1)*KB) of the global order)
            ihx = cpool.tile([128, NB8 * QW], i16, tag="ihx")
            isx = cpool.tile([128, NB8 * QW], i16, tag="isx")
            for k in range(8):
                for b in range(8):
                    csl = slice(b * KB * QW, (b + 1) * KB * QW)
                    nc.sync.dma_start(out=ihx[16 * k : 16 * k + 16, csl], in_=ih_g[16 * b : 16 * b + 16, :])
                    nc.sync.dma_start(out=isx[16 * k : 16 * k + 16, csl], in_=is_g[16 * b : 16 * b + 16, :])
            dl16 = cpool.tile([128, NB8 * B], bf, tag="dl16")
            for b in range(8):
                nc.sync.dma_start(out=dl16[:, b * KB * B : (b + 1) * KB * B], in_=dl_g[128 * b : 128 * (b + 1), :])
            dlf = cpool.tile([128, NB8 * B], f32, tag="dlf")
            nc.vector.tensor_copy(out=dlf[:], in_=dl16[:])

            # ---------------- phase 1: h_ext = [x@W | a_src | a_dst | 1] ----
            with tc.tile_pool(name="ptp", bufs=2, space="PSUM") as ptp:
                for t in range(NT):
                    n0 = t * P
                    nr = min(P, N - n0)
                    if X_INT8:
                        xi8 = xa.tile([P, IN], mybir.dt.int8, tag="xi8")
                        nc.sync.dma_start(out=xi8[:nr, :], in_=x_full[n0 : n0 + nr, :])
                        xsc = xa.tile([P, 1], f32, tag="xsc")
                        nc.sync.dma_start(out=xsc[:nr, :], in_=xs_g[n0 : n0 + nr, :])
                        xc = xa.tile([P, IN], bf, tag="xc")
                        nc.vector.tensor_copy(out=xc[:nr, :], in_=xi8[:nr, :])
                        xn = xa.tile([P, IN], bf, tag="xn")
                        nc.vector.tensor_scalar_mul(out=xn[:nr, :], in0=xc[:nr, :], scalar1=xsc[:nr, 0:1])
                    else:
                        xn = xa.tile([P, IN], bf, tag="xn")
                        nc.sync.dma_start(out=xn[:nr, :], in_=x_full[n0 : n0 + nr, :])
                    pt_ = ptp.tile([P, 2 * P], bf, space="PSUM")
                    nc.tensor.transpose(pt_[:, 0:nr], xn[:nr, 0:P], ident_t[:nr, :nr])
                    nc.tensor.transpose(pt_[:, P : P + nr], xn[:nr, P : 2 * P], ident_t[:nr, :nr])
                    xt = xa.tile([P, 2 * P], bf, tag="xt")
                    nc.vector.tensor_copy(out=xt[:, 0:nr], in_=pt_[:, 0:nr])
                    nc.vector.tensor_copy(out=xt[:, P : P + nr], in_=pt_[:, P : P + nr])
                    ph_t = ph.tile([P, C], f32, space="PSUM")
                    nc.tensor.matmul(out=ph_t[:nr, :], lhsT=xt[:, 0:nr], rhs=w0[:], start=True, stop=False)
                    nc.tensor.matmul(out=ph_t[:nr, :], lhsT=xt[:, P : P + nr], rhs=w1[:], start=False, stop=True)
                    pa_t = pa.tile([P, 2], f32, space="PSUM", tag="pa_t")
                    nc.tensor.matmul(out=pa_t[:nr, :], lhsT=xt[:, 0:nr], rhs=wsd0[:], start=True, stop=False)
                    nc.tensor.matmul(out=pa_t[:nr, :], lhsT=xt[:, P : P + nr], rhs=wsd1[:], start=False, stop=True)
                    h_sb = hs.tile([P, ROW], bf, tag="hsb")
                    nc.vector.tensor_copy(out=h_sb[:nr, 0:C], in_=ph_t[:nr, :])
                    nc.vector.tensor_copy(out=h_sb[:nr, SCOFF : SCOFF + 2], in_=pa_t[:nr, :])
                    nc.vector.memset(h_sb[:nr, SCOFF + 2 : SCOFF + 3], 1.0)
                    nc.sync.dma_start(out=h_ext[n0 : n0 + nr, :], in_=h_sb[:nr, :])
                    sc_sb = hs.tile([P, 128], bf, tag="scsb")
                    nc.vector.tensor_copy(out=sc_sb[:nr, 0:2], in_=pa_t[:nr, :])
                    nc.sync.dma_start(out=sc_tab[n0 : n0 + nr, :], in_=sc_sb[:nr, :])

            tc.strict_bb_all_engine_barrier()

            # ---------------- phase 2: gather / softmax / scatter -----------
            with (
                tc.tile_pool(name="gb", bufs=3) as gb,
                tc.tile_pool(name="scp", bufs=4) as scp,
                tc.tile_pool(name="ohp", bufs=4) as ohp,
                tc.tile_pool(name="po", bufs=4, space="PSUM") as po,
                tc.tile_pool(name="ou", bufs=3) as ou,
            ):
                g_tiles = {}
                e_tiles = {}
                for ev in pp["events"]:
                    if ev[0] == "batch":
                        bi = ev[1]
                        shi, dhi = pp["batches"][bi]
                        c0 = bi * (NIDX // 16)
                        g_t = gb.tile([P, B * ROW], bf, tag="g")
                        s_t = gb.tile([P, B * 128], bf, tag="s")
                        tab = h_ext[HI_OFF:, :] if shi else h_ext[:, :]
                        stab = sc_tab[HI_OFF:, :] if dhi else sc_tab[:, :]
                        QN = 1024
                        for q in range(NIDX // QN):
                            qsl = slice(c0 + q * (QN // 16), c0 + (q + 1) * (QN // 16))
                            gsl = slice(q * (QN // P) * ROW, (q + 1) * (QN // P) * ROW)
                            ssl = slice(q * (QN // P) * 128, (q + 1) * (QN // P) * 128)
                            nc.gpsimd.dma_gather(
                                g_t[:, gsl].rearrange("p (c e) -> p c e", e=ROW),
                                tab, ihx[:, qsl], QN, QN, ROW,
                                single_packet=True,
                            )
                            nc.gpsimd.dma_gather(
                                s_t[:, ssl].rearrange("p (c e) -> p c e", e=128),
                                stab, isx[:, qsl], QN, QN, 128,
                                single_packet=True,
                            )
                        g3 = g_t[:].rearrange("p (c e) -> p c e", e=ROW)
                        s3 = s_t[:].rearrange("p (c e) -> p c e", e=128)
                        ss = scp.tile([P, B], f32, tag="ss")
                        se = scp.tile([P, B], f32, tag="se")
                        nc.vector.tensor_tensor(
                            out=ss[:].rearrange("p (c e) -> p c e", e=1),
                            in0=g3[:, :, SCOFF : SCOFF + 1],
                            in1=s3[:, :, 1:2],
                            op=mybir.AluOpType.add,
                        )
                        nc.scalar.activation(out=ss[:], in_=ss[:], func=mybir.ActivationFunctionType.Prelu, alpha=NEG_SLOPE)
                        nc.scalar.activation(out=se[:], in_=ss[:], func=mybir.ActivationFunctionType.Exp)
                        g_tiles[bi] = g_t
                        e_tiles[bi] = se
                    else:
                        _, t, nr, slots = ev
                        pt = po.tile([P, C + 3], f32, space="PSUM")
                        nch = len(slots)
                        for j, (bi, s) in enumerate(slots):
                            oh_t = ohp.tile([P, P], bf, tag="oh")
                            nc.vector.tensor_scalar(
                                out=oh_t[:],
                                in0=iota_t[:],
                                scalar1=dlf[:, bi * B + s : bi * B + s + 1],
                                scalar2=e_tiles[bi][:, s : s + 1],
                                op0=mybir.AluOpType.is_equal,
                                op1=mybir.AluOpType.mult,
                            )
                            nc.tensor.matmul(
                                out=pt[:, :],
                                lhsT=oh_t[:],
                                rhs=g_tiles[bi][:, s * ROW : s * ROW + C + 3],
                                start=(j == 0),
                                stop=(j == nch - 1),
                            )
                        dn = ou.tile([P, 1], f32, tag="dn")
                        nc.vector.tensor_scalar(
                            out=dn[:], in0=pt[:, C + 2 : C + 3],
                            scalar1=float(H), scalar2=float(H) * EPS,
                            op0=mybir.AluOpType.mult, op1=mybir.AluOpType.add,
                        )
                        rc = ou.tile([P, 1], f32, tag="rc")
                        nc.vector.reciprocal(out=rc[:], in_=dn[:])
                        ob = ou.tile([P, C], f32, tag="ob")
                        nc.vector.tensor_scalar_mul(out=ob[:], in0=pt[:, 0:C], scalar1=rc[:, :1])
                        nc.sync.dma_start(out=rs_in[t * P : t * P + nr, :], in_=ob[:nr, :])

                # ------------ phase 3: ReduceScatter + f16 readback ---------
                tc.strict_bb_all_engine_barrier()
                nc.gpsimd.collective_compute(
                    "ReduceScatter", mybir.AluOpType.add,
                    replica_groups=GRP, ins=[rs_in.opt()], outs=[rs_out.opt()],
                )
                for tt in range(NTS):
                    r0 = tt * P
                    rr = min(P, NS - r0)
                    fb = ou.tile([P, C], f32, tag="fb")
                    nc.sync.dma_start(out=fb[:rr, :], in_=rs_out[r0 : r0 + rr, :])
                    fb16 = ou.tile([P, C], f16, tag="fb16")
                    nc.vector.tensor_copy(out=fb16[:rr, :], in_=fb[:rr, :])
                    nc.sync.dma_start(out=t_out[r0 : r0 + rr, :], in_=fb16[:rr, :])

    nc.finalize()
    return nc


_IOTA = np.broadcast_to(np.arange(P, dtype=np.float32), (P, P)).copy()
_IDENT = np.eye(P, dtype=BF16)

_CACHE = {}


def _get_compiled(edge_index):
    ck = _CACHE.get("edge_index")
    if ck is None or not np.array_equal(ck, edge_index):
        pp = _preprocess(edge_index)
        nc = _build_program(pp)
        _CACHE.update(edge_index=edge_index.copy(), pp=pp, nc=nc, in_key=None)
    return _CACHE["pp"], _CACHE["nc"]


def _make_in_maps(x, W, att_src, att_dst, pp):
    key = _CACHE.get("in_key")
    if key is not None:
        ox, oW, osrc, odst = key
        if (
            np.array_equal(ox, x)
            and np.array_equal(oW, W)
            and np.array_equal(osrc, att_src)
            and np.array_equal(odst, att_dst)
        ):
            return _CACHE["in_maps"]

    if X_INT8:
        absx = np.maximum(np.abs(x).max(axis=1), 1e-20).astype(np.float32)
        xq = np.round(x * (127.0 / absx)[:, None]).astype(np.int8)
        xsc = (absx / 127.0).reshape(N, 1)
    else:
        xq = x.astype(BF16)
    NB = len(pp["batches"])
    KB = (NB + 7) // 8
    NB8 = KB * 8
    QW = NIDX // 16
    idxh = np.zeros((16, NB8 * QW), np.int16)
    idxh[:, : NB * QW] = pp["idxh"]
    idxs = np.zeros((16, NB8 * QW), np.int16)
    idxs[:, : NB * QW] = pp["idxs"]
    dstl = np.full((128, NB8 * B), -1.0, BF16)
    dstl[:, : NB * B] = pp["dstl"]
    base = {"iota": _IOTA, "ident": _IDENT}
    in_maps = []
    for h in range(H):
        Wh = W[:, h * C : (h + 1) * C].astype(np.float32)
        wsrc = Wh @ att_src[h].astype(np.float32)
        wdst = Wh @ att_dst[h].astype(np.float32)
        m = dict(base)
        m["xsl"] = xq[h * NS : (h + 1) * NS, :]
        if X_INT8:
            m["xsc"] = xsc[h * NS : (h + 1) * NS, :]
        m["Wh"] = Wh.astype(BF16)
        m["wsd"] = np.stack([wsrc, wdst], axis=1).astype(BF16)
        m["idxh"] = idxh[:, h * KB * QW : (h + 1) * KB * QW]
        m["idxs"] = idxs[:, h * KB * QW : (h + 1) * KB * QW]
        m["dstl"] = dstl[:, h * KB * B : (h + 1) * KB * B]
        in_maps.append(m)
    _CACHE["in_key"] = (x.copy(), W.copy(), att_src.copy(), att_dst.copy())
    _CACHE["in_maps"] = in_maps
    return in_maps


def kernel(x, edge_index, W, att_src, att_dst, bias, _timing=None):
    x = np.asarray(x)
    edge_index = np.asarray(edge_index)
    W = np.asarray(W)
    att_src = np.asarray(att_src)
    att_dst = np.asarray(att_dst)
    bias = np.asarray(bias)

    pp, nc = _get_compiled(edge_index)
    in_maps = _make_in_maps(x, W, att_src, att_dst, pp)
    res = run_bass_kernel_spmd(nc, in_maps, core_ids=list(range(H)))
    if _timing is not None:
        _timing["exec_time_ns"] = res.exec_time_ns
    out = np.concatenate(
        [res.results[h]["out"] for h in range(H)], axis=0
    ).astype(np.float32)
    out += bias.astype(np.float32)[None, :]
    return out
